# revision 12
# baseline (speedup 1.0000x reference)
"""Trainium2 Bass kernel for the 2-layer GATv2 GNN (nn_GAT_75325136437379).

Self-contained: host-side graph partitioning + SPMD Bass kernel on 8 cores.

Strategy (graph/data parallel, edge-cut by destination):
 - Host relabels nodes into a [320 blocks x 128 slots] grid, degree-balanced
   serpentine, so each core owns 40 blocks (5120 node slots) and every
   dst-block has a near-equal edge count.
 - Edges are assigned to the core owning their destination; per (core, block)
   they are split by source-node half (the dma_gather index is int16, so each
   gather table view is <32768 rows) and padded to fixed K_LO/K_HI slots.
 - On device, each core redundantly computes the dense encoder + per-layer
   feature transform for ALL nodes (channel-major matmuls, no transposes) and
   writes a bf16 node-feature table to DRAM.  The edge phase processes one
   dst-block at a time: gather source rows, build a one-hot selector S via
   is_equal, expand destination rows + add source rows inside PSUM with
   matmuls, leaky-relu (Prelu) -> attention scores -> exp, then segment
   denominator and weighted aggregation as selector matmuls.
 - One AllGather of the post-GAT0 features between the two layers.
 - Decoder fused into the layer-1 edge phase epilogue; edge alphas of layer 1
   are written out and un-permuted on the host.
"""
import sys
import numpy as np

sys.path.insert(0, "/opt/trn_rl_repo")

import ml_dtypes  # noqa: E402
from concourse import bass, bacc, mybir, tile  # noqa: E402
from concourse.bass_utils import run_bass_kernel_spmd  # noqa: E402

bf16 = ml_dtypes.bfloat16
f32 = np.float32

# Problem constants (hardcoded per the task contract).
N, E, IDIM, HLD, H = 40000, 640000, 64, 128, 2
NCORES = 8
NBTOT = 320              # total dst blocks
NB = NBTOT // NCORES     # 40 blocks per core
BLK = 128
NPER = NB * BLK          # 5120 node slots per core
NPAD = NBTOT * BLK       # 40960 node slots total
HALF = NPAD // 2         # 20480: src-half split for int16 gather indices
ROWS_PER_NODE = 125      # 40000 / 320
CH = H * HLD             # 256 feature channels in the transformed table
ZLO_IDX = HALF           # index of the zero row inside the lo table view
ZHI_IDX = HALF           # index of the zero row inside the hi table view
TROWS = NPAD + 2         # table rows: lo | zrow | hi | zrow
RCH = 512                # dense-phase row chunk
NCHUNK = NPAD // RCH     # 80

_BUILD_CACHE = {}
LAST_RESULTS = None      # BassKernelResults of the most recent run (for test.py)


def _ceil_mult(x, m):
    return ((int(x) + m - 1) // m) * m


def _wrap_idx16(arr):
    """[NB, K] int -> gather idx tile [128, NB*K/16] int16 (x8 replicated)."""
    NBb, K = arr.shape
    t = arr.reshape(NBb, K // 16, 16).transpose(2, 0, 1).reshape(16, NBb * (K // 16))
    return np.tile(t.astype(np.int16), (8, 1))


def _wrap_col(arr, dtype):
    """[NB, K] -> slot-wrapped [128, NB*(K/128)] (slot i -> [i%128, i//128])."""
    NBb, K = arr.shape
    return (
        arr.reshape(NBb, K // 128, 128).transpose(2, 0, 1).reshape(128, NBb * (K // 128))
    ).astype(dtype)


def _table_row(g):
    """Global node slot id -> feature-table row (zero row inserted at HALF)."""
    return g + (g >= HALF)


def _preprocess(x, edgeIdx):
    src = edgeIdx[0].astype(np.int64)
    dst = edgeIdx[1].astype(np.int64)

    # ---- node relabeling: degree-balanced serpentine over 320 blocks ----
    deg = np.bincount(dst, minlength=N)
    order = np.argsort(-deg, kind="stable")
    i = np.arange(N)
    rnd = i // NBTOT
    pos = i % NBTOT
    bb = np.where(rnd % 2 == 0, pos, NBTOT - 1 - pos)
    newid = np.empty(N, np.int64)
    newid[order] = bb * BLK + rnd          # rnd < 125 < 128
    src2 = newid[src]
    dst2 = newid[dst]

    core = dst2 // NPER
    blk = (dst2 % NPER) // BLK
    p = dst2 % BLK
    lo = src2 < HALF
    key = core * NB + blk                  # 0..319

    n_lo = np.bincount(key[lo], minlength=NBTOT)
    n_hi = np.bincount(key[~lo], minlength=NBTOT)
    K_LO = max(256, _ceil_mult(n_lo.max(), 128))
    K_HI = max(256, _ceil_mult(n_hi.max(), 128))
    K_FIX = K_LO + K_HI
    CF = K_FIX // 128

    # ---- slot assignment within (core, block, half) ----
    grp = key * 2 + (~lo)                  # lo edges first within a block
    ord_e = np.argsort(grp, kind="stable")
    grp_s = grp[ord_e]
    # rank within group
    starts = np.searchsorted(grp_s, np.arange(NBTOT * 2))
    rank = np.arange(E) - starts[grp_s]
    slot_s = np.where(~(grp_s % 2).astype(bool), rank, K_LO + rank)
    slot = np.empty(E, np.int64)
    slot[ord_e] = slot_s                   # padded slot per original edge

    # ---- gather index + ldst arrays ----
    idx_lo = np.full((NCORES, NB, K_LO), ZLO_IDX, np.int64)
    idx_hi = np.full((NCORES, NB, K_HI), ZHI_IDX, np.int64)
    ldst = np.full((NCORES, NB, K_FIX), 255.0, np.float32)

    el = np.flatnonzero(lo)
    eh = np.flatnonzero(~lo)
    idx_lo[core[el], blk[el], slot[el]] = src2[el]
    idx_hi[core[eh], blk[eh], slot[eh] - K_LO] = src2[eh] - HALF
    ldst[core, blk, slot] = p.astype(np.float32)

    # hb (dst-block rows) gather indices: one gather per table half
    g_dst = (
        np.arange(NCORES)[:, None, None] * NPER
        + np.arange(NB)[None, :, None] * BLK
        + np.arange(BLK)[None, None, :]
    )
    hbA = np.where(g_dst < HALF, g_dst, ZLO_IDX)
    hbB = np.where(g_dst >= HALF, g_dst - HALF, ZHI_IDX)

    per_core = []
    for k in range(NCORES):
        per_core.append(
            dict(
                idx_lo=_wrap_idx16(idx_lo[k]),
                idx_hi=_wrap_idx16(idx_hi[k]),
                idx_hbA=_wrap_idx16(hbA[k].reshape(1, -1)).reshape(128, -1),
                idx_hbB=_wrap_idx16(hbB[k].reshape(1, -1)).reshape(128, -1),
                ldst_col=_wrap_col(ldst[k], f32),
                ldst_row=ldst[k].reshape(1, -1).astype(bf16),
            )
        )

    # ---- permuted, padded, transposed x ----
    xT = np.zeros((IDIM, NPAD), f32)
    xT[:, newid] = np.asarray(x, f32).T

    meta = dict(K_LO=K_LO, K_HI=K_HI, K_FIX=K_FIX, CF=CF, newid=newid,
                alpha_core=core, alpha_blk=blk, alpha_slot=slot)
    return xT.astype(bf16), per_core, meta


def _build_program(CF, K_LO, K_HI):
    key = (CF, K_LO, K_HI)
    if key in _BUILD_CACHE:
        return _BUILD_CACHE[key]
    K_FIX = K_LO + K_HI
    CL = K_LO // 128
    CL16 = K_LO // 16
    CH16 = K_HI // 16
    dt = mybir.dt

    nc = bacc.Bacc("TRN2", target_bir_lowering=False, debug=False,
                   num_devices=NCORES)

    def din(name, shape, dtype):
        return nc.dram_tensor(name, shape, dtype, kind="ExternalInput")

    xT = din("xT", [IDIM, NPAD], dt.bfloat16)
    w_enc0 = din("w_enc0", [IDIM, HLD], dt.bfloat16)
    w_enc1 = din("w_enc1", [HLD, HLD], dt.bfloat16)
    w_gat0 = din("w_gat0", [HLD, CH], dt.bfloat16)
    w_gat1 = din("w_gat1", [HLD, CH], dt.bfloat16)
    w_dec0 = din("w_dec0", [HLD, HLD], dt.bfloat16)
    w_dec1 = din("w_dec1", [HLD, 1], dt.bfloat16)
    b_enc0 = din("b_enc0", [HLD, 1], dt.float32)
    b_enc1 = din("b_enc1", [HLD, 1], dt.float32)
    bl_g0 = din("bl_g0", [128, CH], dt.float32)       # gat0_bl replicated rows
    bl_g1 = din("bl_g1", [128, CH], dt.float32)
    bias_g0 = din("bias_g0", [128, HLD], dt.float32)  # 2*gat0_bias replicated
    bias_g1 = din("bias_g1", [128, HLD], dt.float32)
    b_dec0 = din("b_dec0", [HLD, 1], dt.float32)
    b_dec1 = din("b_dec1", [1, 1], dt.float32)
    att0 = din("att0", [128, CH], dt.bfloat16)        # att rows replicated
    att1 = din("att1", [128, CH], dt.bfloat16)
    iota_row = din("iota_row", [128, 128], dt.float32)
    iota_col = din("iota_col", [128, 1], dt.float32)
    iota_colb = din("iota_colb", [128, 1], dt.bfloat16)
    ident = din("ident", [128, 128], dt.bfloat16)
    alpha02 = din("alpha02", [128, 1], dt.float32)

    idx_lo_t = din("idx_lo", [128, NB * CL16], dt.int16)
    idx_hi_t = din("idx_hi", [128, NB * CH16], dt.int16)
    idx_hbA_t = din("idx_hbA", [128, NB * 8], dt.int16)
    idx_hbB_t = din("idx_hbB", [128, NB * 8], dt.int16)
    ldst_col_t = din("ldst_col", [128, NB * CF], dt.float32)
    ldst_row_t = din("ldst_row", [1, NB * K_FIX], dt.bfloat16)

    y_out = nc.dram_tensor("y_out", [1, NPER], dt.float32, kind="ExternalOutput")
    alpha_out = nc.dram_tensor("alpha_out", [128, NB * CF], dt.float32,
                               kind="ExternalOutput")

    def bc(ap, dims):
        """Insert [0, n] broadcast dims at the given (pos, n) list."""
        lst = [list(d) for d in ap.ap]
        for posn, n in dims:
            lst.insert(posn, [0, n])
        return bass.AP(ap.tensor, ap.offset, lst)

    with tile.TileContext(nc) as tc:
        with (
            tc.tile_pool(name="const", bufs=1) as cp,
            tc.tile_pool(name="dram", bufs=1, space="DRAM") as dpool,
        ):
            # ---- persistent DRAM tables ----
            h0 = dpool.tile([TROWS, CH], dt.bfloat16)
            h1 = dpool.tile([TROWS, CH], dt.bfloat16)
            f1T_loc = dpool.tile([128, NPER], dt.bfloat16)
            f1T_all = dpool.tile([NCORES, 128, NPER], dt.bfloat16)

            # ---- constants into SBUF ----
            def ld(t, shape, dtype):
                s = cp.tile(shape, dtype)
                nc.sync.dma_start(out=s[:], in_=t[:, :] if len(shape) == 2 else t[:])
                return s

            w_enc0_s = ld(w_enc0, [IDIM, HLD], dt.bfloat16)
            w_enc1_s = ld(w_enc1, [HLD, HLD], dt.bfloat16)
            w_gat0_s = ld(w_gat0, [HLD, CH], dt.bfloat16)
            w_gat1_s = ld(w_gat1, [HLD, CH], dt.bfloat16)
            w_dec0_s = ld(w_dec0, [HLD, HLD], dt.bfloat16)
            w_dec1_s = ld(w_dec1, [HLD, 1], dt.bfloat16)
            b_enc0_s = ld(b_enc0, [HLD, 1], dt.float32)
            b_enc1_s = ld(b_enc1, [HLD, 1], dt.float32)
            bl_g0_s = ld(bl_g0, [128, CH], dt.float32)
            bl_g1_s = ld(bl_g1, [128, CH], dt.float32)
            bias_g0_s = ld(bias_g0, [128, HLD], dt.float32)
            bias_g1_s = ld(bias_g1, [128, HLD], dt.float32)
            b_dec0_s = ld(b_dec0, [HLD, 1], dt.float32)
            b_dec1_s = ld(b_dec1, [1, 1], dt.float32)
            att0_s = ld(att0, [128, CH], dt.bfloat16)
            att1_s = ld(att1, [128, CH], dt.bfloat16)
            iota_row_s = ld(iota_row, [128, 128], dt.float32)
            iota_col_s = ld(iota_col, [128, 1], dt.float32)
            iota_colb_s = ld(iota_colb, [128, 1], dt.bfloat16)
            ident_s = ld(ident, [128, 128], dt.bfloat16)
            alpha02_s = ld(alpha02, [128, 1], dt.float32)

            idx_lo_s = ld(idx_lo_t, [128, NB * CL16], dt.int16)
            idx_hi_s = ld(idx_hi_t, [128, NB * CH16], dt.int16)
            idx_hbA_s = ld(idx_hbA_t, [128, NB * 8], dt.int16)
            idx_hbB_s = ld(idx_hbB_t, [128, NB * 8], dt.int16)
            ldst_col_s = ld(ldst_col_t, [128, NB * CF], dt.float32)

            # zero row for both table halves
            zrow = cp.tile([1, CH], dt.bfloat16)
            nc.gpsimd.memset(zrow[:], 0)
            for tab in (h0, h1):
                nc.sync.dma_start(out=tab[HALF:HALF + 1, :], in_=zrow[:])
                nc.sync.dma_start(out=tab[TROWS - 1:TROWS, :], in_=zrow[:])

            # ============ dense phase ============
            def dense_phase(tag, w_gat_s, bl_s, src_kind):
                """Write transformed features into the given table.

                src_kind == 'x': encoder MLP from xT, then gat0 transform.
                src_kind == 'f1': gat1 transform from the all-gathered f1T.
                """
                tab = h0 if src_kind == "x" else h1
                with (
                    tc.tile_pool(name=f"d_sb{tag}", bufs=3) as sb,
                    tc.tile_pool(name=f"d_ps{tag}", bufs=3, space="PSUM") as psp,
                    tc.tile_pool(name=f"d_ps3{tag}", bufs=3, space="PSUM") as psp3,
                ):
                    if src_kind == "x":
                        for c in range(NCHUNK):
                            r0 = c * RCH
                            xt = sb.tile([IDIM, RCH], dt.bfloat16, tag="xt")
                            nc.sync.dma_start(out=xt[:], in_=xT[:, r0:r0 + RCH])
                            ps1 = psp.tile([128, RCH], dt.float32, tag="dps")
                            nc.tensor.matmul(out=ps1[:], lhsT=w_enc0_s[:],
                                             rhs=xt[:], start=True, stop=True)
                            h1t = sb.tile([128, RCH], dt.bfloat16, tag="h1t")
                            nc.scalar.activation(h1t[:], ps1[:],
                                                 mybir.ActivationFunctionType.Silu,
                                                 bias=b_enc0_s[:])
                            ps2 = psp.tile([128, RCH], dt.float32, tag="dps")
                            nc.tensor.matmul(out=ps2[:], lhsT=w_enc1_s[:],
                                             rhs=h1t[:], start=True, stop=True)
                            h2t = sb.tile([128, RCH], dt.bfloat16, tag="h2t")
                            nc.scalar.activation(h2t[:], ps2[:],
                                                 mybir.ActivationFunctionType.Silu,
                                                 bias=b_enc1_s[:])
                            for j in range(RCH // 128):
                                g0 = r0 + j * 128
                                t0 = _table_row(g0)
                                ps3 = psp3.tile([128, CH], dt.float32, tag="dps3")
                                nc.tensor.matmul(out=ps3[:],
                                                 lhsT=h2t[:, j * 128:(j + 1) * 128],
                                                 rhs=w_gat_s[:],
                                                 start=True, stop=True)
                                hh = sb.tile([128, CH], dt.bfloat16, tag="hh")
                                nc.vector.tensor_tensor(out=hh[:], in0=ps3[:],
                                                        in1=bl_s[:],
                                                        op=mybir.AluOpType.add)
                                nc.sync.dma_start(out=tab[t0:t0 + 128, :], in_=hh[:])
                    else:
                        for kk in range(NCORES):
                            for j in range(NB):
                                g0 = kk * NPER + j * 128
                                t0 = _table_row(g0)
                                lh = sb.tile([128, 128], dt.bfloat16, tag="lh")
                                nc.sync.dma_start(
                                    out=lh[:],
                                    in_=f1T_all[kk, :, j * 128:(j + 1) * 128])
                                ps3 = psp3.tile([128, CH], dt.float32, tag="dps3")
                                nc.tensor.matmul(out=ps3[:], lhsT=lh[:],
                                                 rhs=w_gat_s[:],
                                                 start=True, stop=True)
                                hh = sb.tile([128, CH], dt.bfloat16, tag="hh")
                                nc.vector.tensor_tensor(out=hh[:], in0=ps3[:],
                                                        in1=bl_s[:],
                                                        op=mybir.AluOpType.add)
                                nc.sync.dma_start(out=tab[t0:t0 + 128, :], in_=hh[:])

            # ============ edge phase ============
            def edge_phase(layer, tab, att_s, bias_s):
                last = layer == 1
                Q = (CF + 1) // 2
                QS = (K_FIX + 511) // 512
                with (
                    tc.tile_pool(name=f"e_sb{layer}", bufs=2) as sb,
                    tc.tile_pool(name=f"e_hb{layer}", bufs=1) as hbp,
                    tc.tile_pool(name=f"e_msg{layer}", bufs=3, space="PSUM") as msgp,
                    tc.tile_pool(name=f"e_nd{layer}", bufs=2, space="PSUM") as ndp,
                    tc.tile_pool(name=f"e_tp{layer}", bufs=2, space="PSUM") as tpp,
                    (tc.tile_pool(name="e_rex", bufs=1, space="PSUM") if last
                     else _null_ctx()) as rexp,
                ):
                    lo_view = tab[0:HALF + 1, :]
                    hi_view = tab[HALF + 1:TROWS, :]

                    # dst-block rows for the whole phase: two gathers + add
                    hbA = hbp.tile([128, NB, CH], dt.bfloat16)
                    nc.gpsimd.dma_gather(hbA[:], lo_view, idx_hbA_s[:],
                                         NB * 128, NB * 128, CH, single_packet=False)
                    hbB = hbp.tile([128, NB, CH], dt.bfloat16)
                    nc.gpsimd.dma_gather(hbB[:], hi_view, idx_hbB_s[:],
                                         NB * 128, NB * 128, CH, single_packet=False)
                    nc.vector.tensor_tensor(out=hbA[:], in0=hbA[:], in1=hbB[:],
                                            op=mybir.AluOpType.add)
                    hball = hbA

                    _eb = int(__import__('os').environ.get('GAT_EBLOCKS', str(NB)))
                    _elev = int(__import__('os').environ.get('GAT_ELEVEL', '99'))
                    for b in range(_eb):
                        hs = sb.tile([128, CF, CH], dt.bfloat16, tag="hs")
                        nc.gpsimd.dma_gather(
                            hs[:, 0:CL, :], lo_view,
                            idx_lo_s[:, b * CL16:(b + 1) * CL16],
                            K_LO, K_LO, CH, single_packet=False)
                        nc.gpsimd.dma_gather(
                            hs[:, CL:CF, :], hi_view,
                            idx_hi_s[:, b * CH16:(b + 1) * CH16],
                            K_HI, K_HI, CH, single_packet=False)

                        if _elev < 2:
                            continue
                        # one-hot S [128, CF, 128]
                        s_t = sb.tile([128, CF, 128], dt.bfloat16, tag="s_t")
                        ld_ap = ldst_col_s[:, b * CF:(b + 1) * CF]
                        nc.vector.tensor_tensor(
                            out=s_t[:], in0=bc(ld_ap, [(2, 128)]),
                            in1=bc(iota_row_s[:], [(1, CF)]),
                            op=mybir.AluOpType.is_equal)

                        if _elev < 3:
                            continue
                        # S.T [128, CF*128] via ones-matmul broadcast + is_equal
                        ldr = sb.tile([1, K_FIX], dt.bfloat16, tag="ldr")
                        nc.sync.dma_start(
                            out=ldr[:],
                            in_=ldst_row_t[0:1, b * K_FIX:(b + 1) * K_FIX])
                        ldb = sb.tile([128, K_FIX], dt.bfloat16, tag="ldb")
                        nc.gpsimd.partition_broadcast(ldb[:], ldr[:])
                        st_t = sb.tile([128, CF * 128], dt.bfloat16, tag="st_t")
                        nc.vector.tensor_tensor(
                            out=st_t[:], in0=ldb[:],
                            in1=iota_colb_s[:].to_broadcast([128, K_FIX]),
                            op=mybir.AluOpType.is_equal)

                        if _elev < 4:
                            continue
                        # messages: lrelu(hd_expand + hs) per chunk pair
                        lr = sb.tile([128, CF, CH], dt.bfloat16, tag="lr")
                        for q in range(Q):
                            nj = min(2, CF - q * 2)
                            mps = msgp.tile([128, 2, CH], dt.float32, tag="msg")
                            for jj in range(nj):
                                j = q * 2 + jj
                                nc.tensor.matmul(
                                    out=mps[:, jj, :],
                                    lhsT=st_t[:, j * 128:(j + 1) * 128],
                                    rhs=hball[:, b, :],
                                    start=True, stop=False)
                                nc.tensor.matmul(
                                    out=mps[:, jj, :],
                                    lhsT=ident_s[:],
                                    rhs=hs[:, j, :],
                                    start=False, stop=True)
                            nc.scalar.activation(
                                lr[:, q * 2:q * 2 + nj, :].rearrange(
                                    "p a b -> p (a b)"),
                                mps[:, 0:nj, :].rearrange("p a b -> p (a b)"),
                                mybir.ActivationFunctionType.Prelu,
                                alpha=alpha02_s[:])

                        if _elev < 5:
                            continue
                        # scores -> exp
                        mul_v = lr[:].rearrange("p a (h c) -> p a h c", h=H)
                        nc.vector.tensor_tensor(
                            out=mul_v,
                            in0=mul_v,
                            in1=bc(att_s[:].rearrange("p (h c) -> p h c", h=H),
                                   [(1, CF)]),
                            op=mybir.AluOpType.mult)
                        sc = sb.tile([128, CF * H], dt.float32, tag="sc")
                        nc.vector.tensor_reduce(
                            out=sc[:], in_=mul_v,
                            axis=mybir.AxisListType.X, op=mybir.AluOpType.add)
                        ex_f = sb.tile([128, CF * H], dt.float32, tag="ex_f")
                        nc.scalar.activation(ex_f[:], sc[:],
                                             mybir.ActivationFunctionType.Exp)
                        ex_b = sb.tile([128, CF, H], dt.bfloat16, tag="ex_b")
                        nc.vector.tensor_copy(
                            out=ex_b[:],
                            in_=ex_f[:].rearrange("p (a h) -> p a h", h=H))

                        if _elev < 6:
                            continue
                        # weighted aggregation + denominator in one matmul chain:
                        # val[:, j] = [ex*hs | ex] so nd = S.T @ val gives both
                        nd = ndp.tile([128, CH + H], dt.float32, tag="nd")
                        val = sb.tile([128, CF, CH + H], dt.bfloat16, tag="val")
                        nc.vector.tensor_tensor(
                            out=bass.AP(val[:].tensor, val[:].offset,
                                        [val[:].ap[0], [CH + H, CF], [HLD, H], [1, HLD]]),
                            in0=hs[:].rearrange("p a (h c) -> p a h c", h=H),
                            in1=bc(ex_b[:], [(3, HLD)]),
                            op=mybir.AluOpType.mult)
                        nc.vector.tensor_copy(
                            out=val[:, :, CH:CH + H],
                            in_=ex_f[:].rearrange("p (a h) -> p a h", h=H))
                        for j in range(CF):
                            nc.tensor.matmul(out=nd[:],
                                             lhsT=s_t[:, j, :],
                                             rhs=val[:, j, :],
                                             start=(j == 0), stop=(j == CF - 1))

                        if _elev < 7:
                            continue
                        den = sb.tile([128, H], dt.float32, tag="den")
                        nc.vector.tensor_scalar_add(den[:], nd[:, CH:CH + H], 1e-16)
                        rden = sb.tile([128, H], dt.float32, tag="rden")
                        nc.vector.reciprocal(rden[:], den[:])

                        if last:
                            rdh = sb.tile([128, H], dt.bfloat16, tag="rdh")
                            nc.vector.tensor_scalar_mul(rdh[:], rden[:], 0.5)
                            rex = rexp.tile([128, CF, H], dt.float32, tag="rex")
                            for j in range(CF):
                                nc.tensor.matmul(out=rex[:, j, :],
                                                 lhsT=st_t[:, j * 128:(j + 1) * 128],
                                                 rhs=rdh[:],
                                                 start=True, stop=True)
                            am_f = sb.tile([128, CF, H], dt.float32, tag="am_f")
                            nc.vector.tensor_tensor(
                                out=am_f[:],
                                in0=ex_f[:].rearrange("p (a h) -> p a h", h=H),
                                in1=rex[:], op=mybir.AluOpType.mult)
                            am = sb.tile([128, CF], dt.float32, tag="am")
                            nc.vector.tensor_reduce(
                                out=am[:], in_=am_f[:],
                                axis=mybir.AxisListType.X, op=mybir.AluOpType.add)
                            nc.sync.dma_start(
                                out=alpha_out[:, b * CF:(b + 1) * CF], in_=am[:])

                        if _elev < 8:
                            continue
                        # epilogue: mean heads + bias + silu
                        o_t = sb.tile([128, H, HLD], dt.float32, tag="o_t")
                        nc.vector.tensor_tensor(
                            out=o_t[:],
                            in0=nd[:, 0:CH].rearrange("p (h c) -> p h c", h=H),
                            in1=bc(rden[:], [(2, HLD)]),
                            op=mybir.AluOpType.mult)
                        hm = sb.tile([128, HLD], dt.float32, tag="hm")
                        nc.vector.tensor_tensor(out=hm[:], in0=o_t[:, 0, :],
                                                in1=o_t[:, 1, :],
                                                op=mybir.AluOpType.add)
                        hm2 = sb.tile([128, HLD], dt.float32, tag="hm2")
                        nc.vector.tensor_tensor(out=hm2[:], in0=hm[:],
                                                in1=bias_s[:],
                                                op=mybir.AluOpType.add)
                        ob = sb.tile([128, HLD], dt.bfloat16, tag="ob")
                        nc.scalar.activation(ob[:], hm2[:],
                                             mybir.ActivationFunctionType.Silu,
                                             scale=0.5)

                        tp = tpp.tile([128, 128], dt.bfloat16, tag="tp")
                        nc.tensor.transpose(out=tp[:], in_=ob[:],
                                            identity=ident_s[:])
                        obT = sb.tile([128, 128], dt.bfloat16, tag="obT")
                        nc.vector.tensor_copy(out=obT[:], in_=tp[:])

                        if not last:
                            nc.sync.dma_start(
                                out=f1T_loc[:, b * 128:(b + 1) * 128], in_=obT[:])
                        else:
                            zps = tpp.tile([128, 128], dt.float32, tag="tp")
                            nc.tensor.matmul(out=zps[:], lhsT=w_dec0_s[:],
                                             rhs=obT[:], start=True, stop=True)
                            zt = sb.tile([128, 128], dt.bfloat16, tag="zt")
                            nc.scalar.activation(
                                zt[:], zps[:],
                                mybir.ActivationFunctionType.Identity,
                                bias=b_dec0_s[:])
                            yps_full = tpp.tile([128, 128], dt.float32, tag="tp")
                            yps = yps_full[0:1, :]
                            nc.tensor.matmul(out=yps, lhsT=w_dec1_s[:],
                                             rhs=zt[:], start=True, stop=True)
                            ysb = sb.tile([1, 128], dt.float32, tag="ysb")
                            nc.scalar.activation(
                                ysb[:], yps,
                                mybir.ActivationFunctionType.Identity,
                                bias=b_dec1_s[:])
                            nc.sync.dma_start(
                                out=y_out[0:1, b * 128:(b + 1) * 128], in_=ysb[:])

            dense_phase("0", w_gat0_s, bl_g0_s, "x")
            edge_phase(0, h0, att0_s, bias_g0_s)
            nc.gpsimd.collective_compute(
                "AllGather", mybir.AluOpType.bypass,
                replica_groups=[list(range(NCORES))],
                ins=[f1T_loc[:].opt()], outs=[f1T_all[:].opt()])
            dense_phase("1", w_gat1_s, bl_g1_s, "f1")
            edge_phase(1, h1, att1_s, bias_g1_s)

    nc.compile()
    _BUILD_CACHE[key] = nc
    return nc


class _null_ctx:
    def __enter__(self):
        return None

    def __exit__(self, *a):
        return False


def kernel(**inputs):
    global LAST_RESULTS
    x = np.asarray(inputs["x"], f32)
    edgeIdx = np.asarray(inputs["edgeIdx"])

    xT, per_core, meta = _preprocess(x, edgeIdx)
    CF, K_LO, K_HI = meta["CF"], meta["K_LO"], meta["K_HI"]
    nc = _build_program(CF, K_LO, K_HI)

    in_maps = _make_in_maps(inputs, xT, per_core)

    res = run_bass_kernel_spmd(nc, in_maps, core_ids=list(range(NCORES)))
    LAST_RESULTS = res

    newid = meta["newid"]
    y_full = np.empty((N, 1), f32)
    ys = np.stack([res.results[k]["y_out"][0] for k in range(NCORES)])  # [8, NPER]
    y_full[:, 0] = ys[newid // NPER, newid % NPER]

    ac, ab, asl = meta["alpha_core"], meta["alpha_blk"], meta["alpha_slot"]
    als = np.stack([res.results[k]["alpha_out"] for k in range(NCORES)])
    alpha_full = als[ac, asl % 128, ab * CF + asl // 128].astype(f32)

    return y_full, alpha_full


def _make_in_maps(inputs, xT, per_core):
    g = lambda n: np.asarray(inputs[n], f32)
    att0_np = g("gat0_att")   # [H, HLD]
    att1_np = g("gat1_att")
    shared = dict(
        xT=xT,
        w_enc0=g("enc_w0").astype(bf16),
        w_enc1=g("enc_w1").astype(bf16),
        w_gat0=g("gat0_W").astype(bf16),
        w_gat1=g("gat1_W").astype(bf16),
        w_dec0=g("dec_w0").astype(bf16),
        w_dec1=g("dec_w1").astype(bf16),
        b_enc0=g("enc_b0").reshape(HLD, 1),
        b_enc1=g("enc_b1").reshape(HLD, 1),
        bl_g0=np.tile(g("gat0_bl").reshape(1, CH), (128, 1)),
        bl_g1=np.tile(g("gat1_bl").reshape(1, CH), (128, 1)),
        bias_g0=np.tile(2.0 * g("gat0_bias").reshape(1, HLD), (128, 1)),
        bias_g1=np.tile(2.0 * g("gat1_bias").reshape(1, HLD), (128, 1)),
        b_dec0=g("dec_b0").reshape(HLD, 1),
        b_dec1=g("dec_b1").reshape(1, 1),
        att0=np.tile(att0_np.reshape(1, CH), (128, 1)).astype(bf16),
        att1=np.tile(att1_np.reshape(1, CH), (128, 1)).astype(bf16),
        iota_row=np.tile(np.arange(128, dtype=f32), (128, 1)),
        iota_col=np.arange(128, dtype=f32).reshape(128, 1),
        iota_colb=np.arange(128, dtype=f32).reshape(128, 1).astype(bf16),
        ident=np.eye(128, dtype=f32).astype(bf16),
        alpha02=np.full((128, 1), 0.2, f32),
    )
    return [dict(shared, **pc) for pc in per_core]
